# revision 18
# baseline (speedup 1.0000x reference)
"""Trainium2 Bass kernel for nn_CFC_Reformer (CFC + Reformer attention block).

Contract: kernel(**inputs) takes the FULL inputs (x: [8,256,96,96] f32 plus
small conv/attention params), shards x along batch across 8 NeuronCores
(pure data parallel, params replicated), runs one fused Bass/Tile program
per core, and gathers the full [8,128,96,96] f32 output.

Key algebraic simplification: the reference's LSH bucket argmax/argsort
gather applies the SAME permutation of the first NH=8 PSP tokens to both k
and v, and softmax attention is invariant under a shared key/value
permutation -- so ctx depends only on tokens 0..7 (the first 8 cells of
the 6x6 pooling grid, rows 0-32 of xr). The whole bucket/sort pipeline and
the other 42 PSP tokens are dead code and are not computed.

Per-core pipeline (one image [256,96,96], all-bf16 PE path):
  A. xr = SiLU(BN(conv3x3(x))) in 1024-col windows (18 bf16 matmuls each);
     after window 3, the 8 token sums t8 (rows 0-32) are reduced and three
     tiny matmuls fold w_k/w_q/w_v into kwT/vs_g/sbias; s = kwT^T @ xr is
     computed behind the conv and parked in SBUF (fp32).
  B. exp sweep: e = exp(s + sbias) in wide ACT ops (one LUT switch).
  C. per 1024-window: z = ones @ e (PE), rcp = approx_fast(1/z) (DVE),
     e2 = e * rcp (bf16), ctx = vs_g^T @ e2 (PE), drain to SBUF.
  D. y = SiLU(BN(w_la1 @ ctx)); mask = tanh(conv3x3(y, w_la2)) via the
     3-shift replica packing; out = ctx*(1+mask) + xr (silu/tanh share one
     ACT LUT set).

Spatial layout on chip: width padded 96 -> 98 (one zero col each side) so
3x3 conv taps are pure column offsets into the flattened row-major image.
"""

import numpy as np
import ml_dtypes

import concourse.bass as bass
import concourse.bacc as bacc
import concourse.mybir as mybir
import concourse.tile as tile
from concourse.bass_utils import run_bass_kernel_spmd

F32 = mybir.dt.float32
F32R = mybir.dt.float32r
BF16 = mybir.dt.bfloat16
FP8 = mybir.dt.float8e4
DR = mybir.MatmulPerfMode.DoubleRow
AF = mybir.ActivationFunctionType
ALU = mybir.AluOpType
AX = mybir.AxisListType

# Problem shapes (hardcoded per the harness contract).
B, H, W = 8, 96, 96
CIN, COUT, QD, NH = 256, 128, 32, 8
LA_MID = 16
EPS = 1e-5
WP = 98                     # padded row width (1 zero col each side)
NPIX = H * WP               # 9408 padded pixels
AW = 1024                   # conv / attention window (bf16 moving max)
NAW = (NPIX + AW - 1) // AW             # 10 windows (9x1024 + 192)
DW = 512                    # phase D psum window
TCOLS = 4 * WP              # row-aligned tile for phase D
NT = NPIX // TCOLS          # 24 spatial tiles
XTOT = 1 + 98 * WP + 8      # host-padded x: guard col + 98 padded rows + slack
Y_G = WP + 2                # y top guard (one padded row + margin)
Y_SZ = Y_G + NPIX + Y_G + 6 # y tile with top/bottom zero guards

_BUILD_CACHE = {}


def _round_fp32r(a):
    """Round-to-nearest-even to fp32r (e8m13) so PE truncation is exact."""
    u = np.ascontiguousarray(a, np.float32).view(np.uint32)
    r = (u + 0x1FF + ((u >> 10) & 1)) & np.uint32(0xFFFFFC00)
    return r.view(np.float32)


def _bf16(a):
    return np.ascontiguousarray(np.asarray(a, np.float32)).astype(
        ml_dtypes.bfloat16)


def _host_prep(inp):
    """Fold BN into conv weights, pre-fold w_q into the token path, and lay
    every parameter out as the SBUF tiles expect ([partition, free],
    contraction on partitions)."""
    f = np.float32
    w_red = np.asarray(inp["w_red"], f)
    binv = np.asarray(inp["bng"], f) / np.sqrt(np.asarray(inp["bnv"], f) + EPS)
    bnbias = np.asarray(inp["bnb"], f) - np.asarray(inp["bnm"], f) * binv
    wf = w_red * binv[:, None, None, None]          # [COUT, CIN, 3, 3]

    w1t = np.empty((128, 2304), f)
    for kc in range(2):
        for dy in range(3):
            for dx in range(3):
                t = kc * 9 + dy * 3 + dx
                # [ci_local, co]
                w1t[:, t * 128:(t + 1) * 128] = wf[:, kc * 128:(kc + 1) * 128, dy, dx].T

    w_la2 = np.asarray(inp["w_la2"], f)             # [COUT, LA_MID, 3, 3]
    # K-packed: partitions (dx-shift s)*32 + ci (32-stride for legal engine
    # partition starts; odd half zero), one matmul per dy tap
    wla2t = np.zeros((96, 3 * 128), f)
    for dy in range(3):
        for sft in range(3):
            wla2t[sft * 32:sft * 32 + LA_MID, dy * 128:(dy + 1) * 128] = \
                w_la2[:, :, dy, sft].T

    F8 = ml_dtypes.float8_e4m3
    lasc = np.asarray(inp["lag"], f) / np.sqrt(np.asarray(inp["lav"], f) + EPS)
    labi = np.asarray(inp["lab"], f) - np.asarray(inp["lam"], f) * lasc

    w_q = np.asarray(inp["w_q"], f)
    b_q = np.asarray(inp["b_q"], f)
    w_k = np.asarray(inp["w_k"], f)
    b_k = np.asarray(inp["b_k"], f)
    w_v = np.asarray(inp["w_v"], f)
    b_v = np.asarray(inp["b_v"], f)

    # token path folded weights (tokens = t8sum/256):
    #   kwT = (w_q^T w_k / 256) @ t8sum + w_q^T b_k   (lhsT layout = A^T)
    #   vs  = t8sum^T @ (w_v^T/256) + b_v
    #   sbias = t8sum^T @ (w_k^T b_q / 256) + b_k.b_q
    AT = w_k.T @ w_q / 256.0                         # [c', c] = A^T
    wvTf = w_v.T / 256.0                             # [c', c]
    u1 = (w_k.T @ b_q / 256.0).reshape(128, 1)
    abias = (w_q.T @ b_k).reshape(128, 1)
    c0v = np.full((NH, 1), float(b_k @ b_q), f)

    return {
        "w1t": _bf16(w1t),
        "bnbias": np.ascontiguousarray(bnbias.reshape(128, 1)),
        "AT": _bf16(AT),
        "wvTf": _bf16(wvTf),
        "u1": _bf16(u1),
        "abias": np.ascontiguousarray(abias),
        "c0v": c0v,
        "bv8": np.ascontiguousarray(np.tile(b_v, (NH, 1))),           # [8,128]
        "lasc": np.ascontiguousarray(lasc.reshape(LA_MID, 1)),
        "labi": np.ascontiguousarray(labi.reshape(LA_MID, 1)),
        "wla1T": _bf16(np.asarray(inp["w_la1"], f).T),                # [128,16]
        "wla2t": _bf16(wla2t),
        "ones88": _bf16(np.ones((NH, NH), f)),
        "ones8c": _bf16(np.ones((NH, 128), f)),
        "iv16": np.full((128, 1), 1.0 / 16.0, f),
    }


# Small parameters ride in two packed tensors (one f32, one bf16): one DMA
# each instead of ~12, and the startup queue stays clear for the x stream.
F32_PACK = [
    ("bnbias", (128, 1)), ("abias", (128, 1)), ("c0v", (NH, 1)),
    ("bv8", (NH, 128)), ("lasc", (LA_MID, 1)), ("labi", (LA_MID, 1)),
    ("iv16", (128, 1)),
]
BF16_PACK = [
    ("wla1T", (128, LA_MID)), ("wla2t", (96, 3 * 128)), ("ones88", (NH, NH)),
    ("ones8c", (NH, 128)),
    ("AT", (128, 128)), ("wvTf", (128, 128)), ("u1", (128, 1)),
]


def _pack_cols(entries):
    out, off = {}, 0
    for name, shape in entries:
        out[name] = off
        off += shape[1]
    return out, off


F32_OFF, F32_N = _pack_cols(F32_PACK)
BF16_OFF, BF16_N = _pack_cols(BF16_PACK)


def _build_packs(w):
    fpack = np.zeros((128, F32_N), np.float32)
    for name, shape in F32_PACK:
        fpack[:shape[0], F32_OFF[name]:F32_OFF[name] + shape[1]] = w.pop(name)
    bpack = np.zeros((128, BF16_N), ml_dtypes.bfloat16)
    for name, shape in BF16_PACK:
        bpack[:shape[0], BF16_OFF[name]:BF16_OFF[name] + shape[1]] = w.pop(name)
    w["fpack"] = fpack
    w["bpack"] = bpack
    return w


# x load chunks (cols of the padded image): first small chunk lets the PE
# start early, later chunks stream in under compute.
XCHUNKS = [(0, 1280), (1280, 5376), (5376, XTOT)]


def build_program(reps=1):
    """Build the single-core SPMD Bass/Tile program. Same program runs on all
    8 cores; only the 'x' input differs per core."""
    nc = bacc.Bacc("TRN2", target_bir_lowering=False, debug=False)

    di = {}
    di["x"] = nc.dram_tensor("x", [128, 2 * XTOT], BF16,
                         kind="ExternalInput").ap()
    di["w1t"] = nc.dram_tensor("w1t", [128, 2304], BF16,
                               kind="ExternalInput").ap()
    di["fpack"] = nc.dram_tensor("fpack", [128, F32_N], F32,
                                 kind="ExternalInput").ap()
    di["bpack"] = nc.dram_tensor("bpack", [128, BF16_N], BF16,
                                 kind="ExternalInput").ap()

    out_d = nc.dram_tensor("out", [COUT, NPIX], BF16,
                           kind="ExternalOutput").ap()

    with tile.TileContext(nc) as tc:
      # one long-lived pool holds every persistent tile (unique tag = own slot)
      with tc.tile_pool(name="perm", bufs=1) as perm:
        def ptile(name, shape, dt=F32):
            return perm.tile(list(shape), dt, name=name, tag=name)

        xk = ptile("xk", [128, 2 * XTOT], BF16)
        xr = ptile("xr", [128, NPIX], BF16)
        s_s = ptile("s_s", [NH, NPIX])
        e_s = ptile("e_s", [NH, NPIX], BF16)
        ctx_s = ptile("ctx_s", [128, NPIX], BF16)
        y_s = ptile("y_s", [96, Y_SZ], BF16)
        rowsum = ptile("rowsum", [128, 192])
        t8sum = ptile("t8sum", [128, NH], BF16)
        kwT_s = ptile("kwT_s", [128, NH], BF16)
        vs_g = ptile("vs_g", [NH, 128], BF16)
        sbias = ptile("sbias", [NH, 1])

        sb = {}
        # critical-path first: conv weights + the first x chunk, then the
        # packed parameter tiles, then the rest of x
        sb["w1t"] = ptile("sb_w1t", [128, 2304], BF16)
        nc.sync.dma_start(out=sb["w1t"][:, 0:1152], in_=di["w1t"][:, 0:1152])
        def load_x(c0, c1):
            # both kc-halves of a column chunk in one DMA (3D APs)
            nc.sync.dma_start(
                out=xk[:, :].rearrange("p (k c) -> p k c", k=2)[:, :, c0:c1],
                in_=di["x"][:, :].rearrange("p (k c) -> p k c", k=2)[:, :, c0:c1])

        load_x(*XCHUNKS[0])
        nc.sync.dma_start(out=sb["w1t"][:, 1152:2304],
                          in_=di["w1t"][:, 1152:2304])
        fpk = ptile("fpk", [128, F32_N], F32)
        nc.sync.dma_start(out=fpk[:, :], in_=di["fpack"][:, :])
        bpk = ptile("bpk", [128, BF16_N], BF16)
        nc.sync.dma_start(out=bpk[:, :], in_=di["bpack"][:, :])

        for name, shape in F32_PACK:
            sb[name] = fpk[0:shape[0],
                           F32_OFF[name]:F32_OFF[name] + shape[1]]
        for name, shape in BF16_PACK:
            sb[name] = bpk[0:shape[0],
                           BF16_OFF[name]:BF16_OFF[name] + shape[1]]
        for c0, c1 in XCHUNKS[1:]:
            load_x(c0, c1)

        # one-time zero of the whole y tile: the 32-stride packing leaves
        # unused partitions whose wla2t rows are zero, but 0 * garbage-inf
        # would poison psum (engine APs need 32-aligned partition starts,
        # so zero everything once)
        nc.gpsimd.memset(y_s[:, :], 0.0)

        for _rep in range(reps):
            # ===== Phase A: conv3x3+BN+SiLU -> xr; tokens + s behind it =====
            def conv_window(apool, w):
                c0 = w * AW
                L = min(AW, NPIX - c0)
                ps = apool.tile([128, AW], F32, tag="apsum")
                # matmul outputs must stay inside one 2KB psum bank -> 512
                for h0 in range(0, L, DW):
                    hL = min(DW, L - h0)
                    for kc in range(2):
                        for dy in range(3):
                            for dx in range(3):
                                t = kc * 9 + dy * 3 + dx
                                off = c0 + h0 + dy * WP + dx
                                nc.tensor.matmul(
                                    ps[:, h0:h0 + hL],
                                    sb["w1t"][:, t * 128:(t + 1) * 128],
                                    xk[:, kc * XTOT + off:kc * XTOT + off + hL],
                                    start=(t == 0), stop=(t == 17))
                nc.scalar.activation(xr[:, c0:c0 + L], ps[:, 0:L], AF.Silu,
                                     bias=sb["bnbias"][:, 0:1])

            def emit_s(spool, w):
                # 512-wide s sub-windows (one psum bank each)
                for half in range(2):
                    c0 = w * AW + half * DW
                    if c0 >= NPIX:
                        return
                    L = min(DW, NPIX - c0)
                    sp = spool.tile([NH, DW], F32, tag="spsum")
                    nc.tensor.matmul(sp[:, 0:L], kwT_s[:, :],
                                     xr[:, c0:c0 + L], start=True, stop=True)
                    nc.vector.tensor_copy(s_s[:, c0:c0 + L], sp[:, 0:L])

            with tc.tile_pool(name="apsum", bufs=3, space="PSUM") as apool:
                for w in range(4):
                    conv_window(apool, w)

                # tokens: 8 means of 16x16 blocks over rows 0-32 (= first 8
                # cells of the reference's 6x6 PSP grid; the only live ones)
                xrb = xr[:, 0:32 * WP].rearrange(
                    "p (r c) -> p r c", c=WP)[:, :, 1:1 + W].rearrange(
                    "p r (j u) -> p r j u", u=16)
                nc.vector.tensor_reduce(
                    rowsum[:, :].rearrange("p (r j) -> p r j", j=6),
                    xrb, axis=AX.X, op=ALU.add)
                with nc.allow_low_precision(reason="bf16 token sums"):
                    nc.vector.tensor_reduce(
                        t8sum[:, 0:6],
                        rowsum[:, 0:96].rearrange("p (r j) -> p j r", j=6),
                        axis=AX.X, op=ALU.add)
                    nc.vector.tensor_reduce(
                        t8sum[:, 6:8],
                        rowsum[:, 96:192].rearrange("p (r j) -> p j r", j=6)[:, 0:2, :],
                        axis=AX.X, op=ALU.add)

                with tc.tile_pool(name="bpsum", bufs=1, space="PSUM") as bpool:
                    # all three tiny outputs share one psum bank
                    bt = bpool.tile([128, 512], F32, tag="b1")
                    kw_p = bt[:, 0:NH]
                    nc.tensor.matmul(kw_p, sb["AT"][:, :],
                                     t8sum[:, :], start=True, stop=True)
                    with nc.allow_low_precision(reason="bf16 kwT"):
                        nc.vector.tensor_scalar(
                            kwT_s[:, :], kw_p, sb["abias"][:, 0:1], None,
                            op0=ALU.add)
                    vs_p = bt[0:NH, 128:256]
                    nc.tensor.matmul(vs_p, t8sum[:, :],
                                     sb["wvTf"][:, :],
                                     start=True, stop=True)
                    with nc.allow_low_precision(reason="bf16 v tokens"):
                        nc.vector.tensor_tensor(vs_g[:, :], vs_p,
                                                sb["bv8"][:, :], op=ALU.add)
                    sb_p = bt[0:NH, 384:385]
                    nc.tensor.matmul(sb_p, t8sum[:, :],
                                     sb["u1"][:, :],
                                     start=True, stop=True)
                    nc.vector.tensor_tensor(sbias[:, :], sb_p,
                                            sb["c0v"][:, :], op=ALU.add)

                with tc.tile_pool(name="spsum", bufs=2, space="PSUM") as spool:
                    for w in range(4, NAW):
                        conv_window(apool, w)
                        emit_s(spool, w - 4)
                    for w in range(NAW - 4, NAW):
                        emit_s(spool, w)

            # ===== Fused tail: attention windows + local-atten pipeline ====
            # Per 512-window: z = ones@e (PE) -> rcp = approx(1/z) (DVE) ->
            # e2 = e*rcp (GpSimd) -> ctx = vs^T@e2 (PE) -> drain (DVE).
            # Per 392-tile as ctx lands: y matmul (PE); per pair: one silu
            # (ACT) + two column-shifted replicas via SBUF->SBUF DMA (Sync
            # engines are otherwise idle); mask conv (PE) + tanh (ACT);
            # blends split DVE/GpSimd; DMA out per 4-tile group.
            NW = (NPIX + DW - 1) // DW          # 19 (18x512 + 192)
            DG = 4
            grp = {}

            def emit_z(w):
                # z replicated onto all 128 partitions so the softmax scale
                # folds into the ctx psum drain (one DVE op, no e2 pass)
                c0 = w * DW
                L = min(DW, NPIX - c0)
                z_p = ps_z.tile([128, DW], F32, tag="z")
                nc.tensor.matmul(z_p[:, 0:L], sb["ones8c"][:, :],
                                 e_s[:, c0:c0 + L], start=True, stop=True)
                grp[("z", w)] = z_p

            def emit_norm(w):
                c0 = w * DW
                L = min(DW, NPIX - c0)
                z_p = grp.pop(("z", w))
                rc_t = cpool.tile([128, DW], F32, tag="rc")
                nc.vector.reciprocal_approx_fast(rc_t[:, 0:L], z_p[:, 0:L])
                grp[("rc", w)] = rc_t

            def emit_ctx(w):
                c0 = w * DW
                L = min(DW, NPIX - c0)
                rc_t = grp.pop(("rc", w))
                ctx_p = ps_ctx.tile([128, DW], F32, tag="ctx")
                nc.tensor.matmul(ctx_p[:, 0:L], vs_g[:, :],
                                 e_s[:, c0:c0 + L], start=True, stop=True)
                with nc.allow_low_precision(reason="bf16 softmax scale"):
                    nc.vector.tensor_tensor(ctx_s[:, c0:c0 + L],
                                            ctx_p[:, 0:L], rc_t[:, 0:L],
                                            op=ALU.mult)

            def emit_y(t):
                # y matmuls for tiles t-1, t land in the two banks of one
                # psum tile; a single 4-level-AP silu drains both (emitted on
                # odd t only); dx-shift replicas via SBUF->SBUF DMA
                c0 = t * TCOLS
                gi = t % 2
                if gi == 0:
                    grp["yp"] = ps_yp.tile([LA_MID, 2 * DW], F32,
                                           name="ypp", tag="yp")
                nc.tensor.matmul(grp["yp"][:, gi * DW:gi * DW + TCOLS],
                                 sb["wla1T"][:, :],
                                 ctx_s[:, c0:c0 + TCOLS], start=True, stop=True)
                if gi == 0:
                    return
                pv = grp["yp"][:, :].rearrange(
                    "p (g c) -> p g c", c=DW)[:, :, 0:TCOLS].rearrange(
                    "p g (r c) -> p g r c", c=WP)[:, :, :, 1:1 + W]
                b2 = Y_G + (t - 1) * TCOLS
                dst1 = y_s[32:32 + LA_MID, b2:b2 + 2 * TCOLS].rearrange(
                    "p (g rc) -> p g rc", g=2).rearrange(
                    "p g (r c) -> p g r c", c=WP)[:, :, :, 1:1 + W]
                nc.scalar.activation(dst1, pv, AF.Silu,
                                     bias=sb["labi"][:, 0:1],
                                     scale=sb["lasc"][:, 0:1])
                src2 = y_s[32:32 + LA_MID, b2:b2 + 2 * TCOLS].rearrange(
                    "p (r c) -> p r c", c=WP)[:, :, 1:1 + W]
                for g, p0, eng in ((0, 0, nc.gpsimd), (2, 64, nc.vector)):
                    dstg = y_s[p0:p0 + LA_MID,
                               b2 + (1 - g):b2 + (1 - g) + 2 * TCOLS].rearrange(
                        "p (r c) -> p r c", c=WP)[:, :, 1:1 + W]
                    eng.tensor_copy(dstg, src2)

            def emit_mask_pair(i):
                m_p = dps.tile([128, 2 * DW], F32, tag="m")
                for h, t in ((0, 2 * i), (1, 2 * i + 1)):
                    c0 = t * TCOLS
                    for dy in range(3):
                        off = Y_G + c0 + (dy - 1) * WP
                        nc.tensor.matmul(
                            m_p[:, h * DW:h * DW + TCOLS],
                            sb["wla2t"][:, dy * 128:(dy + 1) * 128],
                            y_s[:, off:off + TCOLS],
                            start=(dy == 0), stop=(dy == 2))
                gi = (2 * i) % DG
                if gi == 0:
                    grp["msk"] = dpool.tile([128, DG * TCOLS], BF16,
                                            name="mskg", tag="msk")
                msrc = m_p[:, :].rearrange(
                    "p (g c) -> p g c", c=DW)[:, :, 0:TCOLS]
                mdst = grp["msk"][:, gi * TCOLS:(gi + 2) * TCOLS].rearrange(
                    "p (g c) -> p g c", c=TCOLS)
                nc.scalar.activation(mdst, msrc, AF.Tanh)
                if gi == DG - 2:
                    g0 = (2 * i - DG + 2) * TCOLS
                    GL = DG * TCOLS
                    ct = dpool.tile([128, GL], BF16, tag="ct")
                    with nc.allow_low_precision(reason="bf16 mask product"):
                        # ct = (msk + 1) * ctx
                        nc.vector.scalar_tensor_tensor(
                            ct[:, :], grp["msk"][:, :], 1.0,
                            ctx_s[:, g0:g0 + GL], op0=ALU.add, op1=ALU.mult)
                    o_t = dpool.tile([128, GL], BF16, tag="o")
                    with nc.allow_low_precision(reason="bf16 output"):
                        nc.gpsimd.tensor_tensor(o_t[:, :], ct[:, :],
                                                xr[:, g0:g0 + GL], op=ALU.add)
                    nc.sync.dma_start(out=out_d[:, g0:g0 + GL], in_=o_t[:, :])

            with (
                tc.tile_pool(name="cpool", bufs=3) as cpool,
                tc.tile_pool(name="dpool", bufs=2) as dpool,
                tc.tile_pool(name="cps_z", bufs=2, space="PSUM") as ps_z,
                tc.tile_pool(name="cps_ctx", bufs=2, space="PSUM") as ps_ctx,
                tc.tile_pool(name="cps_yp", bufs=1, space="PSUM") as ps_yp,
                tc.tile_pool(name="dpsum", bufs=1, space="PSUM") as dps,
            ):
                # exp sweep (one LUT switch after the conv silus)
                for a in range(0, NPIX, 3200):
                    b = min(a + 3200, NPIX)
                    nc.scalar.activation(e_s[:, a:b], s_s[:, a:b], AF.Exp,
                                         bias=sbias[:, 0:1])

                # software pipeline over 512-windows; D-tile work keyed to
                # drained-ctx progress. Mask pair i needs y silus through
                # tile 2i+2 (halo rows), y tile t needs ctx cols (t+1)*392.
                done_ctx = 0
                t_y = 0
                i_mask = 0

                def drain_d():
                    nonlocal t_y, i_mask
                    while t_y < NT and (t_y + 1) * TCOLS <= done_ctx:
                        emit_y(t_y)
                        t_y += 1
                    # pair i needs y tiles through 2i+2 silu'd; silu for tile
                    # t lands when emit_y(t) with odd t ran (covers t-1, t)
                    while i_mask < NT // 2 and (
                            t_y >= 2 * i_mask + 4 or t_y == NT):
                        emit_mask_pair(i_mask)
                        i_mask += 1

                for w in range(NW):
                    emit_z(w)
                    if w >= 1:
                        emit_norm(w - 1)
                    if w >= 2:
                        emit_ctx(w - 2)
                        done_ctx = (w - 1) * DW
                        drain_d()
                emit_norm(NW - 1)
                emit_ctx(NW - 2)
                emit_ctx(NW - 1)
                done_ctx = NPIX
                drain_d()

    nc.compile()
    return nc


def get_program(reps=1):
    key = ("nc", reps)
    if key not in _BUILD_CACHE:
        _BUILD_CACHE[key] = build_program(reps)
    return _BUILD_CACHE[key]


def pad_x(xb):
    """[256,96,96] -> zero-framed, kc-interleaved [128, 2*XTOT] bf16."""
    xp = np.zeros((CIN, XTOT), np.float32)
    body = np.zeros((CIN, 98, WP), np.float32)
    body[:, 1:1 + H, 1:1 + W] = xb
    xp[:, 1:1 + 98 * WP] = body.reshape(CIN, 98 * WP)
    out = np.empty((128, 2 * XTOT), np.float32)
    out[:, 0:XTOT] = xp[0:128]
    out[:, XTOT:2 * XTOT] = xp[128:256]
    return out.astype(ml_dtypes.bfloat16)


def unpad_out(flat):
    """[128, 9408] padded rows -> [128, 96, 96]."""
    return np.ascontiguousarray(flat.reshape(COUT, H, WP)[:, :, 1:1 + W])


def kernel(**inputs):
    x = np.ascontiguousarray(np.asarray(inputs["x"], np.float32))
    assert x.shape == (B, CIN, H, W)
    weights = _build_packs(_host_prep(inputs))
    nc = get_program()
    in_maps = [dict(weights, x=pad_x(x[b])) for b in range(B)]
    res = run_bass_kernel_spmd(nc, in_maps, list(range(B)))
    out = np.stack([unpad_out(res.results[b]["out"]) for b in range(B)], axis=0)
    return out.astype(np.float32)


# revision 19
# speedup vs baseline: 1.2439x; 1.2439x over previous
"""Trainium2 Bass kernel for nn_CFC_Reformer (CFC + Reformer attention block).

Contract: kernel(**inputs) takes the FULL inputs (x: [8,256,96,96] f32 plus
small conv/attention params), shards x along batch across 8 NeuronCores
(pure data parallel, params replicated), runs one fused Bass/Tile program
per core, and gathers the full [8,128,96,96] f32 output.

Key algebraic simplification: the reference's LSH bucket argmax/argsort
gather applies the SAME permutation of the first NH=8 PSP tokens to both k
and v, and softmax attention is invariant under a shared key/value
permutation -- so ctx depends only on tokens 0..7 (the first 8 cells of
the 6x6 pooling grid, rows 0-32 of xr). The whole bucket/sort pipeline and
the other 42 PSP tokens are dead code and are not computed.

Per-core pipeline (one image [256,96,96], all-bf16 PE path):
  A. xr = SiLU(BN(conv3x3(x))) in 1024-col windows (18 bf16 matmuls each);
     after window 3, the 8 token sums t8 (rows 0-32) are reduced and three
     tiny matmuls fold w_k/w_q/w_v into kwT/vs_g/sbias; s = kwT^T @ xr is
     computed behind the conv and parked in SBUF (fp32).
  B. exp sweep: e = exp(s + sbias) in wide ACT ops (one LUT switch).
  C. per 1024-window: z = ones @ e (PE), rcp = approx_fast(1/z) (DVE),
     e2 = e * rcp (bf16), ctx = vs_g^T @ e2 (PE), drain to SBUF.
  D. y = SiLU(BN(w_la1 @ ctx)); mask = tanh(conv3x3(y, w_la2)) via the
     3-shift replica packing; out = ctx*(1+mask) + xr (silu/tanh share one
     ACT LUT set).

Spatial layout on chip: width padded 96 -> 98 (one zero col each side) so
3x3 conv taps are pure column offsets into the flattened row-major image.
"""

import numpy as np
import ml_dtypes

import concourse.bass as bass
import concourse.bacc as bacc
import concourse.mybir as mybir
import concourse.tile as tile
from concourse.bass_utils import run_bass_kernel_spmd

F32 = mybir.dt.float32
F32R = mybir.dt.float32r
BF16 = mybir.dt.bfloat16
FP8 = mybir.dt.float8e4
DR = mybir.MatmulPerfMode.DoubleRow
AF = mybir.ActivationFunctionType
ALU = mybir.AluOpType
AX = mybir.AxisListType

# Problem shapes (hardcoded per the harness contract).
B, H, W = 8, 96, 96
CIN, COUT, QD, NH = 256, 128, 32, 8
LA_MID = 16
EPS = 1e-5
WP = 98                     # padded row width (1 zero col each side)
NPIX = H * WP               # 9408 padded pixels
AW = 1024                   # conv / attention window (bf16 moving max)
NAW = (NPIX + AW - 1) // AW             # 10 windows (9x1024 + 192)
DW = 512                    # phase D psum window
TCOLS = 4 * WP              # row-aligned tile for phase D
NT = NPIX // TCOLS          # 24 spatial tiles
XTOT = 1 + 98 * WP + 8      # host-padded x: guard col + 98 padded rows + slack
Y_G = WP + 2                # y top guard (one padded row + margin)
Y_SZ = Y_G + NPIX + Y_G + 6 # y tile with top/bottom zero guards

_BUILD_CACHE = {}


def _round_fp32r(a):
    """Round-to-nearest-even to fp32r (e8m13) so PE truncation is exact."""
    u = np.ascontiguousarray(a, np.float32).view(np.uint32)
    r = (u + 0x1FF + ((u >> 10) & 1)) & np.uint32(0xFFFFFC00)
    return r.view(np.float32)


def _bf16(a):
    return np.ascontiguousarray(np.asarray(a, np.float32)).astype(
        ml_dtypes.bfloat16)


def _host_prep(inp):
    """Fold BN into conv weights, pre-fold w_q into the token path, and lay
    every parameter out as the SBUF tiles expect ([partition, free],
    contraction on partitions)."""
    f = np.float32
    w_red = np.asarray(inp["w_red"], f)
    binv = np.asarray(inp["bng"], f) / np.sqrt(np.asarray(inp["bnv"], f) + EPS)
    bnbias = np.asarray(inp["bnb"], f) - np.asarray(inp["bnm"], f) * binv
    wf = w_red * binv[:, None, None, None]          # [COUT, CIN, 3, 3]

    w1t = np.empty((128, 2304), f)
    for kc in range(2):
        for dy in range(3):
            for dx in range(3):
                t = kc * 9 + dy * 3 + dx
                # [ci_local, co]
                w1t[:, t * 128:(t + 1) * 128] = wf[:, kc * 128:(kc + 1) * 128, dy, dx].T

    w_la2 = np.asarray(inp["w_la2"], f)             # [COUT, LA_MID, 3, 3]
    # K-packed: partitions (dx-shift s)*32 + ci (32-stride for legal engine
    # partition starts; odd half zero), one matmul per dy tap
    wla2t = np.zeros((96, 3 * 128), f)
    for dy in range(3):
        for sft in range(3):
            wla2t[sft * 32:sft * 32 + LA_MID, dy * 128:(dy + 1) * 128] = \
                w_la2[:, :, dy, sft].T

    F8 = ml_dtypes.float8_e4m3
    lasc = np.asarray(inp["lag"], f) / np.sqrt(np.asarray(inp["lav"], f) + EPS)
    labi = np.asarray(inp["lab"], f) - np.asarray(inp["lam"], f) * lasc

    w_q = np.asarray(inp["w_q"], f)
    b_q = np.asarray(inp["b_q"], f)
    w_k = np.asarray(inp["w_k"], f)
    b_k = np.asarray(inp["b_k"], f)
    w_v = np.asarray(inp["w_v"], f)
    b_v = np.asarray(inp["b_v"], f)

    # token path folded weights (tokens = t8sum/256):
    #   kwT = (w_q^T w_k / 256) @ t8sum + w_q^T b_k   (lhsT layout = A^T)
    #   vs  = t8sum^T @ (w_v^T/256) + b_v
    #   sbias = t8sum^T @ (w_k^T b_q / 256) + b_k.b_q
    AT = w_k.T @ w_q / 256.0                         # [c', c] = A^T
    wvTf = w_v.T / 256.0                             # [c', c]
    u1 = (w_k.T @ b_q / 256.0).reshape(128, 1)
    abias = (w_q.T @ b_k).reshape(128, 1)
    c0v = np.full((NH, 1), float(b_k @ b_q), f)

    return {
        "w1t": _bf16(w1t),
        "bnbias": np.ascontiguousarray(bnbias.reshape(128, 1)),
        "AT": _bf16(AT),
        "wvTf": _bf16(wvTf),
        "u1": _bf16(u1),
        "abias": np.ascontiguousarray(abias),
        "c0v": c0v,
        "bv8": np.ascontiguousarray(np.tile(b_v, (NH, 1))),           # [8,128]
        "lasc": np.ascontiguousarray(lasc.reshape(LA_MID, 1)),
        "labi": np.ascontiguousarray(labi.reshape(LA_MID, 1)),
        "wla1T": _bf16(np.asarray(inp["w_la1"], f).T),                # [128,16]
        "wla2t": _bf16(wla2t),
        "ones88": _bf16(np.ones((NH, NH), f)),
        "ones8c": _bf16(np.ones((NH, 128), f)),
        "iv16": np.full((128, 1), 1.0 / 16.0, f),
    }


# Small parameters ride in two packed tensors (one f32, one bf16): one DMA
# each instead of ~12, and the startup queue stays clear for the x stream.
F32_PACK = [
    ("bnbias", (128, 1)), ("abias", (128, 1)), ("c0v", (NH, 1)),
    ("bv8", (NH, 128)), ("lasc", (LA_MID, 1)), ("labi", (LA_MID, 1)),
    ("iv16", (128, 1)),
]
BF16_PACK = [
    ("wla1T", (128, LA_MID)), ("wla2t", (96, 3 * 128)), ("ones88", (NH, NH)),
    ("ones8c", (NH, 128)),
    ("AT", (128, 128)), ("wvTf", (128, 128)), ("u1", (128, 1)),
]


def _pack_cols(entries):
    out, off = {}, 0
    for name, shape in entries:
        out[name] = off
        off += shape[1]
    return out, off


F32_OFF, F32_N = _pack_cols(F32_PACK)
BF16_OFF, BF16_N = _pack_cols(BF16_PACK)


def _build_packs(w):
    fpack = np.zeros((128, F32_N), np.float32)
    for name, shape in F32_PACK:
        fpack[:shape[0], F32_OFF[name]:F32_OFF[name] + shape[1]] = w.pop(name)
    bpack = np.zeros((128, BF16_N), ml_dtypes.bfloat16)
    for name, shape in BF16_PACK:
        bpack[:shape[0], BF16_OFF[name]:BF16_OFF[name] + shape[1]] = w.pop(name)
    w["fpack"] = fpack
    w["bpack"] = bpack
    return w


# x load chunks (cols of the padded image): first small chunk lets the PE
# start early, later chunks stream in under compute.
XCHUNKS = [(0, 1280), (1280, 5376), (5376, XTOT)]


def build_program(reps=1):
    """Build the single-core SPMD Bass/Tile program. Same program runs on all
    8 cores; only the 'x' input differs per core."""
    nc = bacc.Bacc("TRN2", target_bir_lowering=False, debug=False)

    di = {}
    di["x"] = nc.dram_tensor("x", [128, 2 * XTOT], BF16,
                         kind="ExternalInput").ap()
    di["w1t"] = nc.dram_tensor("w1t", [128, 2304], BF16,
                               kind="ExternalInput").ap()
    di["fpack"] = nc.dram_tensor("fpack", [128, F32_N], F32,
                                 kind="ExternalInput").ap()
    di["bpack"] = nc.dram_tensor("bpack", [128, BF16_N], BF16,
                                 kind="ExternalInput").ap()

    out_d = nc.dram_tensor("out", [COUT, NPIX], BF16,
                           kind="ExternalOutput").ap()

    with tile.TileContext(nc) as tc:
      # one long-lived pool holds every persistent tile (unique tag = own slot)
      with tc.tile_pool(name="perm", bufs=1) as perm:
        def ptile(name, shape, dt=F32):
            return perm.tile(list(shape), dt, name=name, tag=name)

        xk = ptile("xk", [128, 2 * XTOT], BF16)
        xr = ptile("xr", [128, NPIX], BF16)
        s_s = ptile("s_s", [NH, NPIX])
        e_s = ptile("e_s", [NH, NPIX], BF16)
        ctx_s = ptile("ctx_s", [128, NPIX], BF16)
        y_s = ptile("y_s", [96, Y_SZ], BF16)
        rowsum = ptile("rowsum", [128, 192])
        t8sum = ptile("t8sum", [128, NH], BF16)
        kwT_s = ptile("kwT_s", [128, NH], BF16)
        vs_g = ptile("vs_g", [NH, 128], BF16)
        sbias = ptile("sbias", [NH, 1])

        sb = {}
        # critical-path first: conv weights + the first x chunk, then the
        # packed parameter tiles, then the rest of x
        sb["w1t"] = ptile("sb_w1t", [128, 2304], BF16)
        nc.sync.dma_start(out=sb["w1t"][:, 0:1152], in_=di["w1t"][:, 0:1152])
        def load_x(c0, c1):
            # both kc-halves of a column chunk in one DMA (3D APs)
            nc.sync.dma_start(
                out=xk[:, :].rearrange("p (k c) -> p k c", k=2)[:, :, c0:c1],
                in_=di["x"][:, :].rearrange("p (k c) -> p k c", k=2)[:, :, c0:c1])

        load_x(*XCHUNKS[0])
        nc.sync.dma_start(out=sb["w1t"][:, 1152:2304],
                          in_=di["w1t"][:, 1152:2304])
        fpk = ptile("fpk", [128, F32_N], F32)
        nc.sync.dma_start(out=fpk[:, :], in_=di["fpack"][:, :])
        bpk = ptile("bpk", [128, BF16_N], BF16)
        nc.sync.dma_start(out=bpk[:, :], in_=di["bpack"][:, :])

        for name, shape in F32_PACK:
            sb[name] = fpk[0:shape[0],
                           F32_OFF[name]:F32_OFF[name] + shape[1]]
        for name, shape in BF16_PACK:
            sb[name] = bpk[0:shape[0],
                           BF16_OFF[name]:BF16_OFF[name] + shape[1]]
        for c0, c1 in XCHUNKS[1:]:
            load_x(c0, c1)

        # one-time zero of the whole y tile: the 32-stride packing leaves
        # unused partitions whose wla2t rows are zero, but 0 * garbage-inf
        # would poison psum (engine APs need 32-aligned partition starts,
        # so zero everything once)
        nc.gpsimd.memset(y_s[:, :], 0.0)

        for _rep in range(reps):
            # ===== Phase A: conv3x3+BN+SiLU -> xr; tokens + s behind it =====
            def conv_window(apool, w):
                c0 = w * AW
                L = min(AW, NPIX - c0)
                ps = apool.tile([128, AW], F32, tag="apsum")
                # matmul outputs must stay inside one 2KB psum bank -> 512
                for h0 in range(0, L, DW):
                    hL = min(DW, L - h0)
                    for kc in range(2):
                        for dy in range(3):
                            for dx in range(3):
                                t = kc * 9 + dy * 3 + dx
                                off = c0 + h0 + dy * WP + dx
                                nc.tensor.matmul(
                                    ps[:, h0:h0 + hL],
                                    sb["w1t"][:, t * 128:(t + 1) * 128],
                                    xk[:, kc * XTOT + off:kc * XTOT + off + hL],
                                    start=(t == 0), stop=(t == 17))
                nc.scalar.activation(xr[:, c0:c0 + L], ps[:, 0:L], AF.Silu,
                                     bias=sb["bnbias"][:, 0:1])

            def emit_s(spool, w):
                # 512-wide s sub-windows (one psum bank each)
                for half in range(2):
                    c0 = w * AW + half * DW
                    if c0 >= NPIX:
                        return
                    L = min(DW, NPIX - c0)
                    sp = spool.tile([NH, DW], F32, tag="spsum")
                    nc.tensor.matmul(sp[:, 0:L], kwT_s[:, :],
                                     xr[:, c0:c0 + L], start=True, stop=True)
                    nc.vector.tensor_copy(s_s[:, c0:c0 + L], sp[:, 0:L])

            with tc.tile_pool(name="apsum", bufs=3, space="PSUM") as apool:
                for w in range(4):
                    conv_window(apool, w)

                # tokens: 8 means of 16x16 blocks over rows 0-32 (= first 8
                # cells of the reference's 6x6 PSP grid; the only live ones)
                xrb = xr[:, 0:32 * WP].rearrange(
                    "p (r c) -> p r c", c=WP)[:, :, 1:1 + W].rearrange(
                    "p r (j u) -> p r j u", u=16)
                nc.vector.tensor_reduce(
                    rowsum[:, :].rearrange("p (r j) -> p r j", j=6),
                    xrb, axis=AX.X, op=ALU.add)
                with nc.allow_low_precision(reason="bf16 token sums"):
                    nc.vector.tensor_reduce(
                        t8sum[:, 0:6],
                        rowsum[:, 0:96].rearrange("p (r j) -> p j r", j=6),
                        axis=AX.X, op=ALU.add)
                    nc.vector.tensor_reduce(
                        t8sum[:, 6:8],
                        rowsum[:, 96:192].rearrange("p (r j) -> p j r", j=6)[:, 0:2, :],
                        axis=AX.X, op=ALU.add)

                with tc.tile_pool(name="bpsum", bufs=1, space="PSUM") as bpool:
                    # all three tiny outputs share one psum bank
                    bt = bpool.tile([128, 512], F32, tag="b1")
                    kw_p = bt[:, 0:NH]
                    nc.tensor.matmul(kw_p, sb["AT"][:, :],
                                     t8sum[:, :], start=True, stop=True)
                    with nc.allow_low_precision(reason="bf16 kwT"):
                        nc.vector.tensor_scalar(
                            kwT_s[:, :], kw_p, sb["abias"][:, 0:1], None,
                            op0=ALU.add)
                    vs_p = bt[0:NH, 128:256]
                    nc.tensor.matmul(vs_p, t8sum[:, :],
                                     sb["wvTf"][:, :],
                                     start=True, stop=True)
                    with nc.allow_low_precision(reason="bf16 v tokens"):
                        nc.vector.tensor_tensor(vs_g[:, :], vs_p,
                                                sb["bv8"][:, :], op=ALU.add)
                    sb_p = bt[0:NH, 384:385]
                    nc.tensor.matmul(sb_p, t8sum[:, :],
                                     sb["u1"][:, :],
                                     start=True, stop=True)
                    nc.vector.tensor_tensor(sbias[:, :], sb_p,
                                            sb["c0v"][:, :], op=ALU.add)

                with tc.tile_pool(name="spsum", bufs=2, space="PSUM") as spool:
                    for w in range(4, NAW):
                        conv_window(apool, w)
                        emit_s(spool, w - 4)
                    for w in range(NAW - 4, NAW):
                        emit_s(spool, w)

            # ===== Fused tail: attention windows + local-atten pipeline ====
            # Per 512-window: z = ones@e (PE) -> rcp = approx(1/z) (DVE) ->
            # e2 = e*rcp (GpSimd) -> ctx = vs^T@e2 (PE) -> drain (DVE).
            # Per 392-tile as ctx lands: y matmul (PE); per pair: one silu
            # (ACT) + two column-shifted replicas via SBUF->SBUF DMA (Sync
            # engines are otherwise idle); mask conv (PE) + tanh (ACT);
            # blends split DVE/GpSimd; DMA out per 4-tile group.
            NW = (NPIX + DW - 1) // DW          # 19 (18x512 + 192)
            DG = 4
            grp = {}

            def emit_z(w):
                c0 = w * DW
                L = min(DW, NPIX - c0)
                z_p = ps_z.tile([NH, DW], F32, tag="z")
                nc.tensor.matmul(z_p[:, 0:L], sb["ones88"][:, :],
                                 e_s[:, c0:c0 + L], start=True, stop=True)
                grp[("z", w)] = z_p

            def emit_norm(w):
                c0 = w * DW
                L = min(DW, NPIX - c0)
                z_p = grp.pop(("z", w))
                rc_t = cpool.tile([NH, DW], F32, tag="rc")
                nc.vector.reciprocal_approx_fast(rc_t[:, 0:L], z_p[:, 0:L])
                e2_t = cpool.tile([NH, DW], BF16, tag="e2")
                with nc.allow_low_precision(reason="bf16 softmax scale"):
                    nc.gpsimd.tensor_tensor(
                        e2_t[:, 0:L], e_s[:, c0:c0 + L], rc_t[:, 0:L],
                        op=ALU.mult)
                grp[("e2", w)] = e2_t

            def emit_ctx(w):
                c0 = w * DW
                L = min(DW, NPIX - c0)
                e2_t = grp.pop(("e2", w))
                ctx_p = ps_ctx.tile([128, DW], F32, tag="ctx")
                nc.tensor.matmul(ctx_p[:, 0:L], vs_g[:, :], e2_t[:, 0:L],
                                 start=True, stop=True)
                nc.vector.tensor_copy(ctx_s[:, c0:c0 + L], ctx_p[:, 0:L])

            def emit_y(t):
                # y matmuls for tiles t-1, t land in the two banks of one
                # psum tile; a single 4-level-AP silu drains both (emitted on
                # odd t only); dx-shift replicas via SBUF->SBUF DMA
                c0 = t * TCOLS
                gi = t % 2
                if gi == 0:
                    grp["yp"] = ps_yp.tile([LA_MID, 2 * DW], F32,
                                           name="ypp", tag="yp")
                nc.tensor.matmul(grp["yp"][:, gi * DW:gi * DW + TCOLS],
                                 sb["wla1T"][:, :],
                                 ctx_s[:, c0:c0 + TCOLS], start=True, stop=True)
                if gi == 0:
                    return
                pv = grp["yp"][:, :].rearrange(
                    "p (g c) -> p g c", c=DW)[:, :, 0:TCOLS].rearrange(
                    "p g (r c) -> p g r c", c=WP)[:, :, :, 1:1 + W]
                b2 = Y_G + (t - 1) * TCOLS
                dst1 = y_s[32:32 + LA_MID, b2:b2 + 2 * TCOLS].rearrange(
                    "p (g rc) -> p g rc", g=2).rearrange(
                    "p g (r c) -> p g r c", c=WP)[:, :, :, 1:1 + W]
                nc.scalar.activation(dst1, pv, AF.Silu,
                                     bias=sb["labi"][:, 0:1],
                                     scale=sb["lasc"][:, 0:1])
                src2 = y_s[32:32 + LA_MID, b2:b2 + 2 * TCOLS].rearrange(
                    "p (r c) -> p r c", c=WP)[:, :, 1:1 + W]
                for g, p0 in ((0, 0), (2, 64)):
                    dstg = y_s[p0:p0 + LA_MID,
                               b2 + (1 - g):b2 + (1 - g) + 2 * TCOLS].rearrange(
                        "p (r c) -> p r c", c=WP)[:, :, 1:1 + W]
                    nc.sync.dma_start(out=dstg, in_=src2)

            def emit_mask_pair(i):
                m_p = dps.tile([128, 2 * DW], F32, tag="m")
                for h, t in ((0, 2 * i), (1, 2 * i + 1)):
                    c0 = t * TCOLS
                    for dy in range(3):
                        off = Y_G + c0 + (dy - 1) * WP
                        nc.tensor.matmul(
                            m_p[:, h * DW:h * DW + TCOLS],
                            sb["wla2t"][:, dy * 128:(dy + 1) * 128],
                            y_s[:, off:off + TCOLS],
                            start=(dy == 0), stop=(dy == 2))
                gi = (2 * i) % DG
                if gi == 0:
                    grp["msk"] = dpool.tile([128, DG * TCOLS], BF16,
                                            name="mskg", tag="msk")
                msrc = m_p[:, :].rearrange(
                    "p (g c) -> p g c", c=DW)[:, :, 0:TCOLS]
                mdst = grp["msk"][:, gi * TCOLS:(gi + 2) * TCOLS].rearrange(
                    "p (g c) -> p g c", c=TCOLS)
                nc.scalar.activation(mdst, msrc, AF.Tanh)
                if gi == DG - 2:
                    g0 = (2 * i - DG + 2) * TCOLS
                    GL = DG * TCOLS
                    ct = dpool.tile([128, GL], BF16, tag="ct")
                    with nc.allow_low_precision(reason="bf16 mask product"):
                        # ct = (msk + 1) * ctx
                        nc.vector.scalar_tensor_tensor(
                            ct[:, :], grp["msk"][:, :], 1.0,
                            ctx_s[:, g0:g0 + GL], op0=ALU.add, op1=ALU.mult)
                    o_t = dpool.tile([128, GL], BF16, tag="o")
                    with nc.allow_low_precision(reason="bf16 output"):
                        nc.gpsimd.tensor_tensor(o_t[:, :], ct[:, :],
                                                xr[:, g0:g0 + GL], op=ALU.add)
                    nc.sync.dma_start(out=out_d[:, g0:g0 + GL], in_=o_t[:, :])

            with (
                tc.tile_pool(name="cpool", bufs=3) as cpool,
                tc.tile_pool(name="dpool", bufs=2) as dpool,
                tc.tile_pool(name="cps_z", bufs=2, space="PSUM") as ps_z,
                tc.tile_pool(name="cps_ctx", bufs=2, space="PSUM") as ps_ctx,
                tc.tile_pool(name="cps_yp", bufs=1, space="PSUM") as ps_yp,
                tc.tile_pool(name="dpsum", bufs=1, space="PSUM") as dps,
            ):
                # exp sweep (one LUT switch after the conv silus)
                for a in range(0, NPIX, 3200):
                    b = min(a + 3200, NPIX)
                    nc.scalar.activation(e_s[:, a:b], s_s[:, a:b], AF.Exp,
                                         bias=sbias[:, 0:1])

                # software pipeline over 512-windows; D-tile work keyed to
                # drained-ctx progress. Mask pair i needs y silus through
                # tile 2i+2 (halo rows), y tile t needs ctx cols (t+1)*392.
                done_ctx = 0
                t_y = 0
                i_mask = 0

                def drain_d():
                    nonlocal t_y, i_mask
                    while t_y < NT and (t_y + 1) * TCOLS <= done_ctx:
                        emit_y(t_y)
                        t_y += 1
                    # pair i needs y tiles through 2i+2 silu'd; silu for tile
                    # t lands when emit_y(t) with odd t ran (covers t-1, t)
                    while i_mask < NT // 2 and (
                            t_y >= 2 * i_mask + 4 or t_y == NT):
                        emit_mask_pair(i_mask)
                        i_mask += 1

                for w in range(NW):
                    emit_z(w)
                    if w >= 1:
                        emit_norm(w - 1)
                    if w >= 2:
                        emit_ctx(w - 2)
                        done_ctx = (w - 1) * DW
                        drain_d()
                emit_norm(NW - 1)
                emit_ctx(NW - 2)
                emit_ctx(NW - 1)
                done_ctx = NPIX
                drain_d()

    nc.compile()
    return nc


def get_program(reps=1):
    key = ("nc", reps)
    if key not in _BUILD_CACHE:
        _BUILD_CACHE[key] = build_program(reps)
    return _BUILD_CACHE[key]


def pad_x(xb):
    """[256,96,96] -> zero-framed, kc-interleaved [128, 2*XTOT] bf16."""
    xp = np.zeros((CIN, XTOT), np.float32)
    body = np.zeros((CIN, 98, WP), np.float32)
    body[:, 1:1 + H, 1:1 + W] = xb
    xp[:, 1:1 + 98 * WP] = body.reshape(CIN, 98 * WP)
    out = np.empty((128, 2 * XTOT), np.float32)
    out[:, 0:XTOT] = xp[0:128]
    out[:, XTOT:2 * XTOT] = xp[128:256]
    return out.astype(ml_dtypes.bfloat16)


def unpad_out(flat):
    """[128, 9408] padded rows -> [128, 96, 96]."""
    return np.ascontiguousarray(flat.reshape(COUT, H, WP)[:, :, 1:1 + W])


def kernel(**inputs):
    x = np.ascontiguousarray(np.asarray(inputs["x"], np.float32))
    assert x.shape == (B, CIN, H, W)
    weights = _build_packs(_host_prep(inputs))
    nc = get_program()
    in_maps = [dict(weights, x=pad_x(x[b])) for b in range(B)]
    res = run_bass_kernel_spmd(nc, in_maps, list(range(B)))
    out = np.stack([unpad_out(res.results[b]["out"]) for b in range(B)], axis=0)
    return out.astype(np.float32)
